# revision 1
# baseline (speedup 1.0000x reference)
"""Self-contained kernel for nn_AdditiveAttention (gnn_message_passing).

Edges are processed in shards; node features and MLP weights are
replicated; partial segment_sum aggregates are summed (all-reduce).
The multi-device psum path crashes the axon PJRT worker in this
environment, so the shards run as device-placed jit calls and the
partial aggregates are reduced on host; numpy is the last-resort path.
"""

import numpy as np

N, E, D, H = 50000, 800000, 128, 128
LN_EPS = 1e-3

_WKEYS = (
    "pW1", "pb1", "pW2", "pb2", "pW3", "pb3", "pg1", "pbe1", "pg2", "pbe2",
    "mW1", "mb1", "mW2", "mb2", "mW3", "mb3", "mg1", "mbe1", "mg2", "mbe2",
)


def _kernel_jax(inputs):
    import jax
    import jax.numpy as jnp

    query = jnp.asarray(inputs["query"], jnp.float32)
    memory = jnp.asarray(inputs["memory"], jnp.float32)
    row = jnp.asarray(inputs["row"]).astype(jnp.int32)
    col = jnp.asarray(inputs["col"]).astype(jnp.int32)
    ws = {k: jnp.asarray(inputs[k], jnp.float32) for k in _WKEYS}

    def layer_norm(x, g, b):
        mu = jnp.mean(x, axis=-1, keepdims=True)
        var = jnp.mean(jnp.square(x - mu), axis=-1, keepdims=True)
        return (x - mu) * jax.lax.rsqrt(var + LN_EPS) * g + b

    def mlp3(x, W1, b1, W2, b2, W3, b3, g1, be1, g2, be2):
        h = layer_norm(jax.nn.relu(x @ W1 + b1), g1, be1)
        h = layer_norm(jax.nn.relu(h @ W2 + b2), g2, be2)
        return h @ W3 + b3

    # the fused single graph trips a neuronxcc internal error
    # (DataLocalityOpt on the fused gather), so each stage is its own jit
    j_gather = jax.jit(lambda t, i: t[i])

    def edge_fn(q, m, mh, w):
        # concat-free first layer: [q|m] @ W1 == q @ W1[:D] + m @ W1[D:]
        x = q @ w["pW1"][:D] + m @ w["pW1"][D:] + w["pb1"]
        hh = layer_norm(jax.nn.relu(x), w["pg1"], w["pbe1"])
        hh = layer_norm(jax.nn.relu(hh @ w["pW2"] + w["pb2"]), w["pg2"],
                        w["pbe2"])
        h = jnp.tanh(hh @ w["pW3"] + w["pb3"])[:, 0]
        return h[:, None] * mh

    j_edge = jax.jit(edge_fn)
    j_memhead = jax.jit(
        lambda m, w: mlp3(m, w["mW1"], w["mb1"], w["mW2"], w["mb2"],
                          w["mW3"], w["mb3"], w["mg1"], w["mbe1"],
                          w["mg2"], w["mbe2"]))
    j_segsum = jax.jit(
        lambda v, r: jax.ops.segment_sum(v, r, num_segments=N))
    j_add = jax.jit(lambda a, b: a + b)

    mem_head = j_memhead(memory, ws)
    ES = 100000
    acc = None
    for s in range(0, E, ES):
        r = row[s:s + ES]
        c = col[s:s + ES]
        v = j_edge(j_gather(query, r), j_gather(memory, c),
                   j_gather(mem_head, c), ws)
        part = j_segsum(v, r)
        acc = part if acc is None else j_add(acc, part)
    return np.asarray(acc, np.float32)


def _kernel_numpy(inputs):
    query = np.asarray(inputs["query"], np.float32)
    memory = np.asarray(inputs["memory"], np.float32)
    row = np.asarray(inputs["row"]).astype(np.int64)
    col = np.asarray(inputs["col"]).astype(np.int64)
    w = {k: np.asarray(inputs[k], np.float32) for k in _WKEYS}

    def layer_norm(x, g, b):
        mu = x.mean(-1, keepdims=True)
        var = np.square(x - mu).mean(-1, keepdims=True)
        return (x - mu) / np.sqrt(var + LN_EPS) * g + b

    def mlp3(x, W1, b1, W2, b2, W3, b3, g1, be1, g2, be2):
        h = layer_norm(np.maximum(x @ W1 + b1, 0.0), g1, be1)
        h = layer_norm(np.maximum(h @ W2 + b2, 0.0), g2, be2)
        return h @ W3 + b3

    mem_head = mlp3(memory, w["mW1"], w["mb1"], w["mW2"], w["mb2"],
                    w["mW3"], w["mb3"], w["mg1"], w["mbe1"], w["mg2"],
                    w["mbe2"])
    out = np.zeros((N, H), np.float32)
    B = 100000
    for s in range(0, E, B):
        r = row[s:s + B]
        c = col[s:s + B]
        units = np.concatenate([query[r], memory[c]], axis=-1)
        h = np.tanh(
            mlp3(units, w["pW1"], w["pb1"], w["pW2"], w["pb2"], w["pW3"],
                 w["pb3"], w["pg1"], w["pbe1"], w["pg2"], w["pbe2"])
        )[:, 0]
        np.add.at(out, r, h[:, None] * mem_head[c])
    return out


def kernel(**inputs) -> np.ndarray:
    try:
        return _kernel_jax(inputs)
    except Exception:
        return _kernel_numpy(inputs)

